# revision 1
# baseline (speedup 1.0000x reference)
"""Trainium2 Bass kernel for nn_Diagnet (S=1024, B=64, I=512, H=2048, O=512).

    u = einsum('sbi,hi->sbh', X, W_ih)
    h_{t} = |u_t + hh * h_{t-1}|   (scan over S, only final h needed)
    Y = h_final @ W_ho.T + b_ho

Strategy (8 NeuronCores, data-parallel over batch, 8 batch rows per core):

* H lanes are permuted so hh is sorted descending and split into 16
  chunks of 128.  The recurrence is a contraction with per-lane factor
  a=hh<1, so a chunk whose largest a satisfies a^K < 1e-10 only needs
  the last K steps: the input->hidden GEMM and the scan skip everything
  earlier (this is exact to ~1e-10 relative, far below fp32 noise).
* Within each 64-step block the state is kept pre-scaled as
  m_tau = a^(63-tau) * h.  Then the step is a multiply-free
  m = |m + a^(63-tau) u_t|, applied by a custom fused DVE op
  (out = |in0 + in1|), one instruction per step over all active chunks.
  Entering a block multiplies the state once by a^64.  Scales a^(63-tau)
  fold into the PSUM->SBUF move of the GEMM output (one tensor_tensor
  multiply).  Underflow of a^64 for small-a lanes reproduces the
  truncation automatically, and no overflow is possible (scales <= 1).
* GEMM: X is pre-tiled host-side into [block, i-chunk, 128, (b,t)]
  (contraction dim on partitions), multiplied against host-transposed
  W_ih^T in fp32.  PSUM layout [h, (b,t)] hands each scan step a
  contiguous slice after a fused scale+move to SBUF.
* Final projection: h_final tiles (already [h,b] on chip) are the
  stationary operand against host-transposed W_ho^T; bias added on DVE.
"""

import math
import os

from contextlib import ExitStack

import numpy as np

S, B, I, H, O = 1024, 64, 512, 2048, 512
NCORES = 8
BC = B // NCORES  # 8 batch rows per core
TB = 64  # time block == scan window
NBLK = S // TB  # 16
NCH = H // 128  # 16 h-chunks
LN_TRUNC = 23.03  # a^K <= e^-23 ~ 1e-10 -> truncate

_CACHE = {}


def _register_abs_add():
    import concourse.dve_ops as dve_ops
    from concourse.dve_spec import Spec, Src0, Src1, Zero, maxx, lower
    from concourse.dve_uop import DveOpSpec

    for op in dve_ops.OPS:
        if op.name == "ABS_ADD_ANT":
            return op
    x = Src0 + Src1
    spec = Spec(
        body=maxx(x, Zero - x),
        reference=lambda in0, in1, s0, s1, imm2: np.abs(
            in0.astype(np.float32) + in1.astype(np.float32)
        ),
    )
    row = max(dve_ops._SUB_OPCODE_FOR_NAME.values()) + 1
    assert row < 0x20
    shas = {}
    for ver in ("v3", "v4"):
        s = DveOpSpec(name="ABS_ADD_ANT", opcode=row, uops=lower(spec, ver=ver), rd1_en=True)
        shas[ver] = s.sha(ver)
    op = dve_ops.DveOp("ABS_ADD_ANT", spec, subdim=False, uops_sha=shas)
    dve_ops._SUB_OPCODE_FOR_NAME["ABS_ADD_ANT"] = row
    dve_ops.OPS.append(op)
    dve_ops.CUSTOM_DVE_SPECS["ABS_ADD_ANT"] = spec
    return op


def _make_plan(hh):
    a = np.maximum(np.abs(hh.astype(np.float64)), 1e-30)
    # jax uniform is [0,1); abs is a no-op safeguard.
    perm = np.argsort(-a, kind="stable")
    a_s = a[perm]
    first_block = []
    for g in range(NCH):
        amax = a_s[g * 128]
        if amax >= math.exp(-LN_TRUNC / S):
            kg = S
        else:
            kg = min(S, int(math.ceil(LN_TRUNC / math.log(1.0 / amax))))
        kg = min(S, ((kg + TB - 1) // TB) * TB)
        first_block.append(NBLK - kg // TB)
    # chunks sorted by a desc -> first_block nondecreasing -> active set is
    # always a chunk prefix.
    assert all(
        first_block[g] <= first_block[g + 1] for g in range(NCH - 1)
    ), first_block
    ag = a_s.reshape(NCH, 128).T  # [128, NCH] lane a per chunk
    tau = np.arange(TB)
    sc = ag[:, :, None] ** (TB - 1 - tau)[None, None, :]  # [128, NCH, TB]
    a64 = np.repeat(ag**TB, BC, axis=1)  # [128, NCH*BC]
    return {
        "perm": perm,
        "first_block": tuple(first_block),
        "SC": sc.reshape(128, NCH * TB).astype(np.float32),
        "A64": a64.astype(np.float32),
    }


def _build(first_block, use_f32r):
    import concourse.mybir as mybir
    import concourse.tile as tile
    from concourse import bacc
    from concourse.bass import ds

    ABS_ADD = _register_abs_add()
    f32 = mybir.dt.float32
    gemm_dt = mybir.dt.float32r if use_f32r else f32

    nc = bacc.Bacc("TRN2", target_bir_lowering=False, debug=False, num_devices=NCORES)
    X = nc.dram_tensor("X", [NBLK, I // 128, 128, TB * BC], gemm_dt, kind="ExternalInput").ap()
    WIHT = nc.dram_tensor("WIHT", [I, H], gemm_dt, kind="ExternalInput").ap()
    WHOT = nc.dram_tensor("WHOT", [H, O], f32, kind="ExternalInput").ap()
    BIAS = nc.dram_tensor("BIAS", [BC, O], f32, kind="ExternalInput").ap()
    SC = nc.dram_tensor("SC", [128, NCH * TB], f32, kind="ExternalInput").ap()
    A64 = nc.dram_tensor("A64", [128, NCH * BC], f32, kind="ExternalInput").ap()
    Y = nc.dram_tensor("Y", [BC, O], f32, kind="ExternalOutput").ap()

    NI = I // 128  # 4 i-chunks

    with tile.TileContext(nc) as tc:
        with ExitStack() as ctx:
            consts = ctx.enter_context(tc.tile_pool(name="consts", bufs=1))
            xtpool = ctx.enter_context(tc.tile_pool(name="xt", bufs=3))
            upool = ctx.enter_context(tc.tile_pool(name="ubuf", bufs=1))
            ypool = ctx.enter_context(tc.tile_pool(name="yout", bufs=1))
            gpool = ctx.enter_context(tc.tile_pool(name="gpsum", bufs=int(os.environ.get("DIAG_GP", "4")), space="PSUM"))
            fpool = ctx.enter_context(tc.tile_pool(name="fpsum", bufs=1, space="PSUM"))

            # constants
            wiht = [consts.tile([128, H], gemm_dt, tag=f"wiht{ic}", name=f"wiht{ic}") for ic in range(NI)]
            for ic in range(NI):
                nc.sync.dma_start(wiht[ic][:], WIHT[ds(ic * 128, 128), :])
            sc_t = consts.tile([128, NCH * TB], f32, tag="sc", name="sc_t")
            nc.sync.dma_start(sc_t[:], SC)
            a64_t = consts.tile([128, NCH * BC], f32, tag="a64", name="a64_t")
            nc.sync.dma_start(a64_t[:], A64)
            m_t = consts.tile([128, NCH * BC], f32, tag="state", name="m_t")
            nc.vector.memset(m_t[:], 0.0)

            acts = [sum(1 for fb in first_block if fb <= kb) for kb in range(NBLK)]
            assert all(a >= 1 for a in acts)
            u_tiles = [None] * NBLK

            def produce(kb):
                act = acts[kb]
                # --- load pre-transposed X tiles [i, (b,t)] ---
                xt = []
                for ic in range(NI):
                    xt_ic = xtpool.tile([128, TB * BC], gemm_dt, tag=f"xt{ic}", name=f"xt_{kb}_{ic}")
                    nc.sync.dma_start(xt_ic[:], X[kb, ic])
                    xt.append(xt_ic)
                # u buffer for this block: [128, (tau, active-chunk, b)]
                u_t = upool.tile([128, TB * act * BC], f32, tag=f"u{kb}", name=f"u_{kb}")
                u_tiles[kb] = u_t
                for g in range(act):
                    ps = gpool.tile([128, TB * BC], f32, tag="gp", name=f"gp_{kb}_{g}")
                    for ic in range(NI):
                        nc.tensor.matmul(
                            ps[:],
                            wiht[ic][:, ds(g * 128, 128)],
                            xt[ic][:],
                            start=(ic == 0),
                            stop=(ic == NI - 1),
                        )
                    # scaled move psum->sbuf:
                    # u_t[p, tau*act*BC + g*BC + b] = ps[p, b*TB+tau]*SC[p,g*TB+tau]
                    dst = u_t[:].rearrange("p (t c) -> p t c", t=TB)[
                        :, :, ds(g * BC, BC)
                    ]
                    srcp = ps[:].rearrange("p (b t) -> p t b", b=BC)
                    scl = sc_t[:, ds(g * TB, TB)].broadcast_to([128, TB, BC])
                    nc.vector.tensor_tensor(dst, srcp, scl, mybir.AluOpType.mult)

            def scan(kb):
                act = acts[kb]
                na = act * BC
                u_t = u_tiles[kb]
                nc.gpsimd.tensor_tensor(
                    m_t[:, 0:na], m_t[:, 0:na], a64_t[:, 0:na], mybir.AluOpType.mult
                )
                for tau in range(TB):
                    nc.vector._custom_dve(
                        ABS_ADD,
                        out=m_t[:, 0:na],
                        in0=m_t[:, 0:na],
                        in1=u_t[:, ds(tau * act * BC, na)],
                    )

            LAG = int(os.environ.get("DIAG_LAG", "2"))
            if os.environ.get("DIAG_ORDER", "seq") == "front":
                # front-load the heaviest (latest) blocks' GEMMs to keep the
                # PE dense/warm while the serial scan chain progresses.
                heavy = [NBLK - 1, NBLK - 2]
                order = heavy + [kb for kb in range(NBLK) if kb not in heavy]
            else:
                order = list(range(NBLK))
            scanned = 0
            produced = set()

            def scan_ready_upto(limit):
                nonlocal scanned
                while scanned < limit and scanned in produced:
                    scan(scanned)
                    scanned += 1

            for i, kb in enumerate(order):
                produce(kb)
                produced.add(kb)
                scan_ready_upto(i + 1 - LAG)
            scan_ready_upto(NBLK)
            assert scanned == NBLK

            # --- final projection: Y = h^T @ WHOT + bias ---
            whot = [consts.tile([128, O], f32, tag=f"whot{g}", name=f"whot{g}") for g in range(NCH)]
            for g in range(NCH):
                nc.sync.dma_start(whot[g][:], WHOT[ds(g * 128, 128), :])
            bias_t = ypool.tile([BC, O], f32, tag="bias", name="bias_t")
            nc.sync.dma_start(bias_t[:], BIAS)
            psy = fpool.tile([BC, O], f32, tag="fy", name="psy")
            for g in range(NCH):
                nc.tensor.matmul(
                    psy[:],
                    m_t[:, ds(g * BC, BC)],
                    whot[g][:],
                    start=(g == 0),
                    stop=(g == NCH - 1),
                )
            y_t = ypool.tile([BC, O], f32, tag="y", name="y_t")
            nc.vector.tensor_tensor(y_t[:], psy[:], bias_t[:], mybir.AluOpType.add)
            nc.sync.dma_start(Y, y_t[:])
    nc.compile()
    return nc


def _get_program(first_block, use_f32r):
    key = (first_block, use_f32r, os.environ.get("DIAG_LAG"), os.environ.get("DIAG_GP"), os.environ.get("DIAG_ORDER"))
    if key not in _CACHE:
        _CACHE[key] = _build(first_block, use_f32r)
    return _CACHE[key]


def _round_f32r(x):
    """Round fp32 array to fp32r (s8e11) representable values."""
    u = np.ascontiguousarray(x).view(np.uint32)
    r = ((u.astype(np.uint64) + 0x800) & 0xFFFFF000).astype(np.uint32)
    return r.view(np.float32).reshape(x.shape)


def _ensure_ntff_hook():
    """Provide antenv.axon_hooks (absent in this image) so trace=True works."""
    import sys
    import types

    if "antenv.axon_hooks" in sys.modules:
        return True
    try:
        import antenv

        mod = types.ModuleType("antenv.axon_hooks")
        mod._hook = None

        def set_axon_ntff_profile_hook(h):
            mod._hook = h

        def get_axon_ntff_profile_hook():
            return mod._hook

        mod.set_axon_ntff_profile_hook = set_axon_ntff_profile_hook
        mod.get_axon_ntff_profile_hook = get_axon_ntff_profile_hook
        sys.modules["antenv.axon_hooks"] = mod
        antenv.axon_hooks = mod

        from trn_agent_boot.trn_boot import _ntff_profile_via_ctypes

        hook = _ntff_profile_via_ctypes("/opt/axon/libaxon_pjrt.so")
        mod.set_axon_ntff_profile_hook(hook)
        return hook is not None
    except Exception:
        return False


def kernel(X, W_ih, hh, W_ho, b_ho):
    from concourse import bass_utils

    X = np.asarray(X, dtype=np.float32)
    W_ih = np.asarray(W_ih, dtype=np.float32)
    hh = np.asarray(hh, dtype=np.float32)
    W_ho = np.asarray(W_ho, dtype=np.float32)
    b_ho = np.asarray(b_ho, dtype=np.float32)

    use_f32r = bool(int(os.environ.get("DIAG_F32R", "0")))
    plan = _make_plan(hh)
    perm = plan["perm"]
    nc = _get_program(plan["first_block"], use_f32r)

    wiht = np.ascontiguousarray(W_ih[perm].T)  # [I, H]
    if use_f32r:
        wiht = _round_f32r(wiht)
    whot = np.ascontiguousarray(W_ho[:, perm].T)  # [H, O]
    bias = np.tile(b_ho[None, :], (BC, 1)).astype(np.float32)

    common = {
        "WIHT": wiht,
        "WHOT": whot,
        "BIAS": bias,
        "SC": plan["SC"],
        "A64": plan["A64"],
    }
    in_maps = []
    for m in range(NCORES):
        im = dict(common)
        xm = X[:, m * BC : (m + 1) * BC, :]  # [S, BC, I]
        # device tile layout [NBLK, NI, 128(i), (b, tau)]
        xt = xm.transpose(2, 1, 0).reshape(I // 128, 128, BC, NBLK, TB)
        xt = np.ascontiguousarray(xt.transpose(3, 0, 1, 2, 4)).reshape(
            NBLK, I // 128, 128, TB * BC
        )
        if use_f32r:
            xt = _round_f32r(xt)
        im["X"] = xt
        in_maps.append(im)

    trace = bool(int(os.environ.get("DIAG_TRACE", "0")))
    if trace:
        trace = _ensure_ntff_hook()
    res = None
    for attempt in range(3):
        try:
            res = bass_utils.run_bass_kernel_spmd(
                nc,
                in_maps,
                core_ids=list(range(NCORES)),
                trace=trace,
                tmpdir=os.environ.get("DIAG_TRACE_DIR") or None,
            )
            break
        except Exception:
            if attempt == 2:
                raise
            trace = False  # retry without profiling
    if res.exec_time_ns is not None:
        kernel.last_exec_time_ns = res.exec_time_ns
        kernel.last_mean_exec_time_ns = res.mean_exec_time_ns
    Yfull = np.concatenate([r["Y"] for r in res.results], axis=0)
    return Yfull


kernel.last_exec_time_ns = None
kernel.last_mean_exec_time_ns = None



# revision 5
# speedup vs baseline: 3.3914x; 3.3914x over previous
"""Trainium2 Bass kernel for nn_Diagnet (S=1024, B=64, I=512, H=2048, O=512).

    u = einsum('sbi,hi->sbh', X, W_ih)
    h_t = |u_t + hh * h_{t-1}|   (scan over S, only final h needed)
    Y = h_final @ W_ho.T + b_ho

Strategy (8 NeuronCores, data-parallel over batch, 8 batch rows per core):

* H lanes are permuted so hh is sorted descending and split into 16
  chunks of 128.  With per-lane decay a = hh < 1, a chunk whose largest
  a satisfies a^K < 1e-10 only needs the last K steps (exact to ~1e-10,
  far below fp32 noise): the GEMM and the scan skip everything earlier.
* The scan runs as ONE custom DVE instruction per (chunk, 64-step
  block): a hand-tuned micro-op program (SCANDIAG_ANT) folds
  m_k = |m_{k-1} - x_k * sc_k| along the free dimension at 1 elem/cycle,
  with the 8 batch chains as sub-dimension pages that re-init the fold
  state to 0 at each page boundary.  State is kept pre-scaled
  (m = a^(K-1-t) h, scales folded into the sc stream), so the step is
  multiply-free in the recurrence itself and no inter-block rescale is
  needed.  Block-to-block state is chained by a carry element prepended
  to each page, written by the previous block's scan through a stride-0
  output AP (65 writes to one address; last write = final state).  The
  scan outputs -m so the carry re-seeds exactly: |0 - (-m)| = m.
* GEMM in bf16 (1 PE cycle/row vs 4 for fp32): X and W_ih^T are cast
  host-side.  PSUM drains to SBUF as plain copies on the otherwise-idle
  Activation engine; the scale (and its sign flip) ride the scan's
  second input stream.
* Final projection: h arrives in [128, (chunk, batch)] layout; 16
  accumulating bf16 matmuls against host-transposed-negated W_ho^T
  (negated because the scan hands over -h), bias added on DVE.
"""

import math
import os

from contextlib import ExitStack

import numpy as np

S, B, I, H, O = 1024, 64, 512, 2048, 512
NCORES = 8
BC = B // NCORES  # 8 batch rows per core
TB = 64  # time block
NBLK = S // TB  # 16
NCH = H // 128  # 16 h-chunks
NI = I // 128  # 4 i-chunks
NSLOT = TB + 1  # carry + 64 scan elements per page
LN_TRUNC = 23.03  # a^K <= e^-23 ~ 1e-10 -> truncate

_CACHE = {}


def _register_scandiag():
    """Custom DVE op: per page s of [P, S, N] in0 (in1 broadcast over pages):
        m = 0
        for k: m = |m - in0[p,s,k] * in1[p,k]| ; out[p,s,k] = -m
    Built from lower() of a Spec, with the step uop hand-edited so the
    fold state re-inits to 0 at each page boundary."""
    import concourse.dve_ops as dve_ops
    from concourse.dve_spec import Spec, Src0, Src1, Zero, scan, PageIdx, lower, AluOp
    from concourse.dve_uop import DveOpSpec, AluInp

    name = "SCANDIAG_ANT"
    for op in dve_ops.OPS:
        if op.name == name:
            return op

    body = (Zero - scan(AluOp.ABSOLUTE_DIFF, Src0 * Src1, init=Zero)) + PageIdx(
        Zero, Zero
    )

    def ref(in0, in1, s0, s1, imm2):
        x = in0.astype(np.float32)
        w = in1.astype(np.float32)
        if x.ndim == 2:
            x = x[:, None, :]
        if w.ndim == 3:
            w = w[:, 0, :]
        out = np.empty_like(x)
        for s in range(x.shape[1]):
            m = np.zeros(x.shape[0], dtype=np.float32)
            for k in range(x.shape[2]):
                m = np.abs(m - x[:, s, k] * w[:, k])
                out[:, s, k] = -m
        return out.reshape(in0.shape)

    spec = Spec(body=body, reference=ref)
    row = max(dve_ops._SUB_OPCODE_FOR_NAME.values()) + 1
    assert row < 0x20
    shas = {}
    compiled = {}
    for ver in ("v3", "v4"):
        uops = lower(spec, ver=ver)
        assert len(uops) == 3, (ver, len(uops))
        hits = [
            k
            for k, b in enumerate(uops[2].datapath_config)
            if b.op == AluOp.ABSOLUTE_DIFF and b.alu_src0 == AluInp.CURR_ALU_OUT
        ]
        assert len(hits) == 1, hits
        k = hits[0]
        seed_blk = uops[0].datapath_config[k]
        assert seed_blk.op == AluOp.BYPASS
        uops[2].datapath_config[k].alu_src0 = seed_blk.alu_src0
        s = DveOpSpec(name=name, opcode=row, uops=uops, rd1_en=True)
        shas[ver] = s.sha(ver)
        compiled[ver] = s
    op = dve_ops.DveOp(name, spec, subdim=True, uops_sha=shas)
    for ver in ("v3", "v4"):
        dve_ops._COMPILE_CACHE[(name, ver)] = compiled[ver]
    dve_ops._SUB_OPCODE_FOR_NAME[name] = row
    dve_ops.OPS.append(op)
    dve_ops.CUSTOM_DVE_SPECS[name] = spec
    return op


def _make_plan(hh):
    a = np.maximum(np.abs(hh.astype(np.float64)), 1e-30)
    perm = np.argsort(-a, kind="stable")
    a_s = a[perm]
    first_block = []
    for g in range(NCH):
        amax = a_s[g * 128]
        if amax >= math.exp(-LN_TRUNC / S):
            kg = S
        else:
            kg = min(S, int(math.ceil(LN_TRUNC / math.log(1.0 / amax))))
        kg = min(S, max(TB, ((kg + TB - 1) // TB) * TB))
        first_block.append(NBLK - kg // TB)
    assert all(
        first_block[g] <= first_block[g + 1] for g in range(NCH - 1)
    ), first_block

    # SC stream: one 65-entry segment per active (chunk, block).
    # slot 0 (carry) = 1.0 ; slot 1+tau = -a^(K-1 - ((kb-fb)*TB+tau))
    ag = a_s.reshape(NCH, 128).T  # [128, NCH]
    segs = [(g, kb) for kb in range(NBLK) for g in range(NCH) if first_block[g] <= kb]
    seg_off = {gk: i * NSLOT for i, gk in enumerate(segs)}
    sc = np.zeros((128, len(segs) * NSLOT), dtype=np.float64)
    tau = np.arange(TB)
    for (g, kb), off in seg_off.items():
        kg = (NBLK - first_block[g]) * TB
        e = kg - 1 - ((kb - first_block[g]) * TB + tau)  # [TB] exponents
        sc[:, off] = 1.0
        sc[:, off + 1 : off + NSLOT] = -(ag[:, g : g + 1] ** e[None, :])
    return {
        "perm": perm,
        "first_block": tuple(first_block),
        "SC": sc.astype(np.float32),
        "nseg": len(segs),
        "seg_off": seg_off,
    }


def _build(first_block, nseg, seg_off):
    import concourse.mybir as mybir
    import concourse.tile as tile
    from concourse import bacc
    from concourse.bass import ds

    SCANDIAG = _register_scandiag()
    f32 = mybir.dt.float32
    bf16 = mybir.dt.bfloat16

    nc = bacc.Bacc("TRN2", target_bir_lowering=False, debug=False, num_devices=NCORES)
    X = nc.dram_tensor("X", [NBLK, 128, NI * BC * TB], bf16, kind="ExternalInput").ap()
    WIHT = nc.dram_tensor("WIHT", [I, H], bf16, kind="ExternalInput").ap()
    SC = nc.dram_tensor("SC", [128, nseg * NSLOT], f32, kind="ExternalInput").ap()
    WHOT = nc.dram_tensor("WHOT", [H, O], bf16, kind="ExternalInput").ap()
    BIAS = nc.dram_tensor("BIAS", [BC, O], f32, kind="ExternalInput").ap()
    Y = nc.dram_tensor("Y", [BC, O], f32, kind="ExternalOutput").ap()

    acts = [[g for g in range(NCH) if first_block[g] <= kb] for kb in range(NBLK)]

    with tile.TileContext(nc) as tc:
        with ExitStack() as ctx:
            consts = ctx.enter_context(tc.tile_pool(name="consts", bufs=1))
            xtpool = ctx.enter_context(tc.tile_pool(name="xt", bufs=3))
            upool = ctx.enter_context(tc.tile_pool(name="ubuf", bufs=2))
            ypool = ctx.enter_context(tc.tile_pool(name="yout", bufs=1))
            gpool = ctx.enter_context(
                tc.tile_pool(name="gpsum", bufs=int(os.environ.get("DIAG_GP", "6")), space="PSUM")
            )
            fpool = ctx.enter_context(tc.tile_pool(name="fpsum", bufs=1, space="PSUM"))

            # --- constants ---
            wiht = consts.tile([128, NI * H], bf16, tag="wiht", name="wiht")
            w4 = wiht[:].rearrange("p (i h) -> p i h", i=NI)
            # leading chunk columns first (unblocks the first GEMMs), then rest
            nc.sync.dma_start(w4[:, :, 0:512], WIHT.rearrange("(i p) h -> p i h", p=128)[:, :, 0:512])
            nc.sync.dma_start(w4[:, :, 512:H], WIHT.rearrange("(i p) h -> p i h", p=128)[:, :, 512:H])
            sc_t = consts.tile([128, nseg * NSLOT], f32, tag="sc", name="sc_t")
            nc.sync.dma_start(sc_t[:], SC)
            hbuf = consts.tile([128, NCH * BC], bf16, tag="hbuf", name="hbuf")

            ucur = {}

            def wslice(g, ic):
                return w4[:, ic, ds(g * 128, 128)]

            for kb in range(NBLK):
                # X tiles for this block: one DMA, [128, (ic, b*t)]
                xt = xtpool.tile([128, NI * TB * BC], bf16, tag="xt", name=f"xt_{kb}")
                x3 = xt[:].rearrange("p (i n) -> p i n", i=NI)
                nc.gpsimd.dma_start(xt[:], X[kb])

                if kb == 3:
                    # needed only at the end; issued here to overlap
                    whot = consts.tile([128, NCH * O], bf16, tag="whot", name="whot")
                    nc.sync.dma_start(
                        whot[:].rearrange("p (g o) -> p g o", g=NCH),
                        WHOT.rearrange("(g p) o -> p g o", p=128),
                    )
                    bias_t = ypool.tile([BC, O], f32, tag="bias", name="bias_t")
                    nc.sync.dma_start(bias_t[:], BIAS)

                for g in acts[kb]:
                    ps = gpool.tile([128, TB * BC], f32, tag="gp", name=f"gp_{kb}_{g}")
                    for ic in range(NI):
                        nc.tensor.matmul(
                            ps[:],
                            wslice(g, ic),
                            x3[:, ic],
                            start=(ic == 0),
                            stop=(ic == NI - 1),
                        )
                    if kb == first_block[g]:
                        u_t = upool.tile([128, BC * NSLOT], f32, tag=f"u{g}", name=f"u_{g}_{kb}")
                        nc.gpsimd.memset(
                            u_t[:].rearrange("p (s n) -> p s n", n=NSLOT)[:, :, 0:1], 0.0
                        )
                        ucur[g] = u_t
                    u3 = ucur[g][:].rearrange("p (s n) -> p s n", n=NSLOT)
                    # drain PSUM -> u slots 1..64 (plain copy on Activation)
                    nc.scalar.copy(
                        u3[:, :, 1:NSLOT], ps[:].rearrange("p (s n) -> p s n", s=BC)
                    )
                    scs = sc_t[:, ds(seg_off[(g, kb)], NSLOT)].unsqueeze(1).broadcast_to(
                        [128, BC, NSLOT]
                    )
                    if kb < NBLK - 1:
                        u_nxt = upool.tile(
                            [128, BC * NSLOT], f32, tag=f"u{g}", name=f"u_{g}_{kb + 1}"
                        )
                        out_ap = (
                            u_nxt[:]
                            .rearrange("p (s n) -> p s n", n=NSLOT)[:, :, 0:1]
                            .broadcast_to([128, BC, NSLOT])
                        )
                    else:
                        out_ap = hbuf[:, ds(g * BC, BC)].broadcast_to([128, BC, NSLOT])
                    nc.vector._custom_dve(SCANDIAG, out=out_ap, in0=u3, in1=scs)
                    if kb < NBLK - 1:
                        ucur[g] = u_nxt

            # --- final projection: Y = (-h)^T @ (-W_ho^T) + bias ---
            psy = fpool.tile([BC, O], f32, tag="fy", name="psy")
            wh3 = whot[:].rearrange("p (g o) -> p g o", g=NCH)
            for g in range(NCH):
                nc.tensor.matmul(
                    psy[:],
                    hbuf[:, ds(g * BC, BC)],
                    wh3[:, g],
                    start=(g == 0),
                    stop=(g == NCH - 1),
                )
            y_t = ypool.tile([BC, O], f32, tag="y", name="y_t")
            nc.vector.tensor_tensor(y_t[:], psy[:], bias_t[:], mybir.AluOpType.add)
            nc.sync.dma_start(Y, y_t[:])
    nc.compile()
    return nc


def _get_program(plan):
    key = (plan["first_block"], os.environ.get("DIAG_GP"))
    if key not in _CACHE:
        _CACHE[key] = _build(plan["first_block"], plan["nseg"], plan["seg_off"])
    return _CACHE[key]


def _ensure_ntff_hook():
    """Provide antenv.axon_hooks (absent in this image) so trace=True works."""
    import sys
    import types

    if "antenv.axon_hooks" in sys.modules:
        return True
    try:
        import antenv

        mod = types.ModuleType("antenv.axon_hooks")
        mod._hook = None

        def set_axon_ntff_profile_hook(h):
            mod._hook = h

        def get_axon_ntff_profile_hook():
            return mod._hook

        mod.set_axon_ntff_profile_hook = set_axon_ntff_profile_hook
        mod.get_axon_ntff_profile_hook = get_axon_ntff_profile_hook
        sys.modules["antenv.axon_hooks"] = mod
        antenv.axon_hooks = mod

        from trn_agent_boot.trn_boot import _ntff_profile_via_ctypes

        hook = _ntff_profile_via_ctypes("/opt/axon/libaxon_pjrt.so")
        mod.set_axon_ntff_profile_hook(hook)
        return hook is not None
    except Exception:
        return False


def kernel(X, W_ih, hh, W_ho, b_ho):
    import ml_dtypes
    from concourse import bass_utils

    bf16 = ml_dtypes.bfloat16
    X = np.asarray(X, dtype=np.float32)
    W_ih = np.asarray(W_ih, dtype=np.float32)
    hh = np.asarray(hh, dtype=np.float32)
    W_ho = np.asarray(W_ho, dtype=np.float32)
    b_ho = np.asarray(b_ho, dtype=np.float32)

    plan = _make_plan(hh)
    perm = plan["perm"]
    nc = _get_program(plan)

    wiht = np.ascontiguousarray(W_ih[perm].T).astype(bf16)  # [I, H]
    whot = np.ascontiguousarray(-W_ho[:, perm].T).astype(bf16)  # [H, O], negated
    bias = np.tile(b_ho[None, :], (BC, 1)).astype(np.float32)

    common = {"WIHT": wiht, "WHOT": whot, "BIAS": bias, "SC": plan["SC"]}
    in_maps = []
    for m in range(NCORES):
        im = dict(common)
        xm = X[:, m * BC : (m + 1) * BC, :]  # [S, BC, I]
        # device layout [NBLK, 128(i-within), (ic, b, tau)] — partition-major
        xt = xm.reshape(NBLK, TB, BC, NI, 128).transpose(0, 4, 3, 2, 1)
        im["X"] = (
            np.ascontiguousarray(xt).reshape(NBLK, 128, NI * BC * TB).astype(bf16)
        )
        in_maps.append(im)

    trace = bool(int(os.environ.get("DIAG_TRACE", "0")))
    if trace:
        trace = _ensure_ntff_hook()
    res = None
    for attempt in range(3):
        try:
            res = bass_utils.run_bass_kernel_spmd(
                nc,
                in_maps,
                core_ids=list(range(NCORES)),
                trace=trace,
                tmpdir=os.environ.get("DIAG_TRACE_DIR") or None,
            )
            break
        except Exception:
            if attempt == 2:
                raise
            trace = False  # retry without profiling
    if res.exec_time_ns is not None:
        kernel.last_exec_time_ns = res.exec_time_ns
        kernel.last_mean_exec_time_ns = res.mean_exec_time_ns
    Yfull = np.concatenate([r["Y"] for r in res.results], axis=0)
    return Yfull


kernel.last_exec_time_ns = None
kernel.last_mean_exec_time_ns = None


# revision 11
# speedup vs baseline: 3.6641x; 1.0804x over previous
"""Trainium2 Bass kernel for nn_Diagnet (S=1024, B=64, I=512, H=2048, O=512).

    u = einsum('sbi,hi->sbh', X, W_ih)
    h_t = |u_t + hh * h_{t-1}|   (scan over S, only final h needed)
    Y = h_final @ W_ho.T + b_ho

Strategy (8 NeuronCores, data-parallel over batch, 8 batch rows per core):

* H lanes are permuted so hh is sorted descending and split into 16
  chunks of 128.  With per-lane decay a = hh < 1, a chunk whose largest
  a satisfies a^K < 1e-10 only needs the last K steps (exact to ~1e-10,
  far below fp32 noise): the GEMM and the scan skip everything earlier.
* The scan runs as ONE custom DVE instruction per (chunk, 64-step
  block): a hand-tuned micro-op program (SCANDIAG_ANT) folds
  m_k = |m_{k-1} - x_k * sc_k| along the free dimension at 1 elem/cycle,
  with the 8 batch chains as sub-dimension pages that re-init the fold
  state to 0 at each page boundary.  State is kept pre-scaled
  (m = a^(K-1-t) h, scales folded into the sc stream), so the step is
  multiply-free in the recurrence itself and no inter-block rescale is
  needed.  Block-to-block state is chained by a carry element prepended
  to each page, written by the previous block's scan through a stride-0
  output AP (65 writes to one address; last write = final state).  The
  scan outputs -m so the carry re-seeds exactly: |0 - (-m)| = m.
* GEMM in bf16 (1 PE cycle/row vs 4 for fp32): X and W_ih^T are cast
  host-side.  PSUM drains to SBUF as plain copies on the otherwise-idle
  Activation engine; the scale (and its sign flip) ride the scan's
  second input stream.
* Final projection: h arrives in [128, (chunk, batch)] layout; 16
  accumulating bf16 matmuls against host-transposed-negated W_ho^T
  (negated because the scan hands over -h), bias added on DVE.
"""

import math
import os

from contextlib import ExitStack

import numpy as np

S, B, I, H, O = 1024, 64, 512, 2048, 512
NCORES = 8
BC = B // NCORES  # 8 batch rows per core
TB = 64  # time block
NBLK = S // TB  # 16
NCH = H // 128  # 16 h-chunks
NI = I // 128  # 4 i-chunks
NSLOT = TB + 1  # carry + 64 scan elements per page
LN_TRUNC = 23.03  # a^K <= e^-23 ~ 1e-10 -> truncate

_CACHE = {}


def _register_scandiag():
    """Custom DVE op: per page s of [P, S, N] in0 (in1 broadcast over pages):
        m = 0
        for k: m = |m - in0[p,s,k] * in1[p,k]| ; out[p,s,k] = -m
    Built from lower() of a Spec, with the step uop hand-edited so the
    fold state re-inits to 0 at each page boundary."""
    import concourse.dve_ops as dve_ops
    from concourse.dve_spec import Spec, Src0, Src1, Zero, scan, PageIdx, lower, AluOp
    from concourse.dve_uop import DveOpSpec, AluInp

    name = "SCANDIAG_ANT"
    for op in dve_ops.OPS:
        if op.name == name:
            return op

    body = (Zero - scan(AluOp.ABSOLUTE_DIFF, Src0 * Src1, init=Zero)) + PageIdx(
        Zero, Zero
    )

    def ref(in0, in1, s0, s1, imm2):
        x = in0.astype(np.float32)
        w = in1.astype(np.float32)
        if x.ndim == 2:
            x = x[:, None, :]
        if w.ndim == 3:
            w = w[:, 0, :]
        out = np.empty_like(x)
        for s in range(x.shape[1]):
            m = np.zeros(x.shape[0], dtype=np.float32)
            for k in range(x.shape[2]):
                m = np.abs(m - x[:, s, k] * w[:, k])
                out[:, s, k] = -m
        return out.reshape(in0.shape)

    spec = Spec(body=body, reference=ref)
    row = max(dve_ops._SUB_OPCODE_FOR_NAME.values()) + 1
    assert row < 0x20
    shas = {}
    compiled = {}
    for ver in ("v3", "v4"):
        uops = lower(spec, ver=ver)
        assert len(uops) == 3, (ver, len(uops))
        hits = [
            k
            for k, b in enumerate(uops[2].datapath_config)
            if b.op == AluOp.ABSOLUTE_DIFF and b.alu_src0 == AluInp.CURR_ALU_OUT
        ]
        assert len(hits) == 1, hits
        k = hits[0]
        seed_blk = uops[0].datapath_config[k]
        assert seed_blk.op == AluOp.BYPASS
        uops[2].datapath_config[k].alu_src0 = seed_blk.alu_src0
        s = DveOpSpec(name=name, opcode=row, uops=uops, rd1_en=True)
        shas[ver] = s.sha(ver)
        compiled[ver] = s
    op = dve_ops.DveOp(name, spec, subdim=True, uops_sha=shas)
    for ver in ("v3", "v4"):
        dve_ops._COMPILE_CACHE[(name, ver)] = compiled[ver]
    dve_ops._SUB_OPCODE_FOR_NAME[name] = row
    dve_ops.OPS.append(op)
    dve_ops.CUSTOM_DVE_SPECS[name] = spec
    return op


def _make_plan(hh):
    a = np.maximum(np.abs(hh.astype(np.float64)), 1e-30)
    perm = np.argsort(-a, kind="stable")
    a_s = a[perm]
    first_block = []
    for g in range(NCH):
        amax = a_s[g * 128]
        if amax >= math.exp(-LN_TRUNC / S):
            kg = S
        else:
            kg = min(S, int(math.ceil(LN_TRUNC / math.log(1.0 / amax))))
        kg = min(S, max(TB, ((kg + TB - 1) // TB) * TB))
        first_block.append(NBLK - kg // TB)
    assert all(
        first_block[g] <= first_block[g + 1] for g in range(NCH - 1)
    ), first_block

    # SC stream: one 65-entry segment per active (chunk, block).
    # slot 0 (carry) = 1.0 ; slot 1+tau = -a^(K-1 - ((kb-fb)*TB+tau))
    ag = a_s.reshape(NCH, 128).T  # [128, NCH]
    segs = [(g, kb) for kb in range(NBLK) for g in range(NCH) if first_block[g] <= kb]
    seg_off = {gk: i * NSLOT for i, gk in enumerate(segs)}
    sc = np.zeros((128, len(segs) * NSLOT), dtype=np.float64)
    tau = np.arange(TB)
    for (g, kb), off in seg_off.items():
        kg = (NBLK - first_block[g]) * TB
        e = kg - 1 - ((kb - first_block[g]) * TB + tau)  # [TB] exponents
        sc[:, off] = 1.0
        sc[:, off + 1 : off + NSLOT] = -(ag[:, g : g + 1] ** e[None, :])
    return {
        "perm": perm,
        "first_block": tuple(first_block),
        "SC": sc.astype(np.float32),
        "nseg": len(segs),
        "seg_off": seg_off,
    }


def _build(first_block, nseg, seg_off):
    import concourse.mybir as mybir
    import concourse.tile as tile
    from concourse import bacc
    from concourse.bass import ds

    SCANDIAG = _register_scandiag()
    f32 = mybir.dt.float32
    bf16 = mybir.dt.bfloat16

    nc = bacc.Bacc("TRN2", target_bir_lowering=False, debug=False, num_devices=NCORES)
    X = nc.dram_tensor("X", [NBLK, 128, NI * BC * TB], bf16, kind="ExternalInput").ap()
    WIHT = nc.dram_tensor("WIHT", [I, H], bf16, kind="ExternalInput").ap()
    SC = nc.dram_tensor("SC", [128, nseg * NSLOT], f32, kind="ExternalInput").ap()
    WHOT = nc.dram_tensor("WHOT", [H, O], bf16, kind="ExternalInput").ap()
    BIAS = nc.dram_tensor("BIAS", [BC, O], f32, kind="ExternalInput").ap()
    Y = nc.dram_tensor("Y", [BC, O], f32, kind="ExternalOutput").ap()

    acts = [[g for g in range(NCH) if first_block[g] <= kb] for kb in range(NBLK)]

    with tile.TileContext(nc) as tc:
        with ExitStack() as ctx:
            consts = ctx.enter_context(tc.tile_pool(name="consts", bufs=1))
            xtpool = ctx.enter_context(tc.tile_pool(name="xt", bufs=6))
            upool = ctx.enter_context(tc.tile_pool(name="ubuf", bufs=3))
            ypool = ctx.enter_context(tc.tile_pool(name="yout", bufs=1))
            gpool = ctx.enter_context(
                tc.tile_pool(name="gpsum", bufs=int(os.environ.get("DIAG_GP", "7")), space="PSUM")
            )
            fpool = ctx.enter_context(tc.tile_pool(name="fpsum", bufs=1, space="PSUM"))

            # --- constants ---
            # chunk-0 columns as a small separate tile: blocks 0..fb[1]-1 only
            # touch chunk 0, and this 128KB DMA unblocks the first GEMMs fast.
            wih0 = consts.tile([128, NI * 128], bf16, tag="wih0", name="wih0")
            nc.sync.dma_start(
                wih0[:].rearrange("p (i h) -> p i h", i=NI),
                WIHT.rearrange("(i p) h -> p i h", p=128)[:, :, 0:128],
            )
            sc_t = consts.tile([128, nseg * NSLOT], f32, tag="sc", name="sc_t")
            nc.sync.dma_start(sc_t[:], SC)
            wiht = [
                consts.tile([128, H], bf16, tag=f"wiht{ic}", name=f"wiht{ic}")
                for ic in range(NI)
            ]
            for ic in range(NI):
                nc.sync.dma_start(wiht[ic][:], WIHT[ds(ic * 128, 128), :])
            hbuf = consts.tile([128, NCH * BC], bf16, tag="hbuf", name="hbuf")

            ucur = {}

            def wslice(g, ic):
                if g == 0:
                    return wih0[:].rearrange("p (i h) -> p i h", i=NI)[:, ic]
                return wiht[ic][:, ds(g * 128, 128)]

            for kb in range(NBLK):
                # X tiles for this block: one DMA, [128, (ic, b*t)]
                xt = xtpool.tile([128, NI * TB * BC], bf16, tag="xt", name=f"xt_{kb}")
                x3 = xt[:].rearrange("p (i n) -> p i n", i=NI)
                if kb == 0:
                    # split the first block across 4 DMAs so it lands sooner
                    xk = X[kb].rearrange("p (i n) -> p i n", i=NI)
                    for ic in range(NI):
                        nc.gpsimd.dma_start(x3[:, ic], xk[:, ic])
                else:
                    nc.gpsimd.dma_start(xt[:], X[kb])

                if kb == 3:
                    # needed only at the end; issued here to overlap
                    whot = consts.tile([128, NCH * O], bf16, tag="whot", name="whot")
                    nc.sync.dma_start(
                        whot[:].rearrange("p (g o) -> p g o", g=NCH),
                        WHOT.rearrange("(g p) o -> p g o", p=128),
                    )
                    bias_t = ypool.tile([BC, O], f32, tag="bias", name="bias_t")
                    nc.sync.dma_start(bias_t[:], BIAS)

                for g in acts[kb]:
                    ps = gpool.tile([128, TB * BC], f32, tag="gp", name=f"gp_{kb}_{g}")
                    for ic in range(NI):
                        nc.tensor.matmul(
                            ps[:],
                            wslice(g, ic),
                            x3[:, ic],
                            start=(ic == 0),
                            stop=(ic == NI - 1),
                        )
                    if kb == first_block[g]:
                        u_t = upool.tile([128, BC * NSLOT], f32, tag=f"u{g}", name=f"u_{g}_{kb}")
                        nc.gpsimd.memset(
                            u_t[:].rearrange("p (s n) -> p s n", n=NSLOT)[:, :, 0:1], 0.0
                        )
                        ucur[g] = u_t
                    u3 = ucur[g][:].rearrange("p (s n) -> p s n", n=NSLOT)
                    # drain PSUM -> u slots 1..64 (plain copy on Activation;
                    # GPSIMD cannot read PSUM on TRN2)
                    ps3 = ps[:].rearrange("p (s n) -> p s n", s=BC)
                    nc.scalar.copy(u3[:, :, 1:NSLOT], ps3)
                    scs = sc_t[:, ds(seg_off[(g, kb)], NSLOT)].unsqueeze(1).broadcast_to(
                        [128, BC, NSLOT]
                    )
                    if kb < NBLK - 1:
                        u_nxt = upool.tile(
                            [128, BC * NSLOT], f32, tag=f"u{g}", name=f"u_{g}_{kb + 1}"
                        )
                        out_ap = (
                            u_nxt[:]
                            .rearrange("p (s n) -> p s n", n=NSLOT)[:, :, 0:1]
                            .broadcast_to([128, BC, NSLOT])
                        )
                    else:
                        out_ap = hbuf[:, ds(g * BC, BC)].broadcast_to([128, BC, NSLOT])
                    nc.vector._custom_dve(SCANDIAG, out=out_ap, in0=u3, in1=scs)
                    if kb < NBLK - 1:
                        ucur[g] = u_nxt

            # --- final projection: Y = (-h)^T @ (-W_ho^T) + bias ---
            psy = fpool.tile([BC, O], f32, tag="fy", name="psy")
            wh3 = whot[:].rearrange("p (g o) -> p g o", g=NCH)
            for g in range(NCH):
                nc.tensor.matmul(
                    psy[:],
                    hbuf[:, ds(g * BC, BC)],
                    wh3[:, g],
                    start=(g == 0),
                    stop=(g == NCH - 1),
                )
            y_t = ypool.tile([BC, O], f32, tag="y", name="y_t")
            nc.vector.tensor_tensor(y_t[:], psy[:], bias_t[:], mybir.AluOpType.add)
            nc.sync.dma_start(Y, y_t[:])
    nc.compile()
    return nc


def _get_program(plan):
    key = (plan["first_block"], os.environ.get("DIAG_GP"))
    if key not in _CACHE:
        _CACHE[key] = _build(plan["first_block"], plan["nseg"], plan["seg_off"])
    return _CACHE[key]


def _ensure_ntff_hook():
    """Provide antenv.axon_hooks (absent in this image) so trace=True works."""
    import sys
    import types

    if "antenv.axon_hooks" in sys.modules:
        return True
    try:
        import antenv

        mod = types.ModuleType("antenv.axon_hooks")
        mod._hook = None

        def set_axon_ntff_profile_hook(h):
            mod._hook = h

        def get_axon_ntff_profile_hook():
            return mod._hook

        mod.set_axon_ntff_profile_hook = set_axon_ntff_profile_hook
        mod.get_axon_ntff_profile_hook = get_axon_ntff_profile_hook
        sys.modules["antenv.axon_hooks"] = mod
        antenv.axon_hooks = mod

        from trn_agent_boot.trn_boot import _ntff_profile_via_ctypes

        hook = _ntff_profile_via_ctypes("/opt/axon/libaxon_pjrt.so")
        mod.set_axon_ntff_profile_hook(hook)
        return hook is not None
    except Exception:
        return False


def kernel(X, W_ih, hh, W_ho, b_ho):
    import ml_dtypes
    from concourse import bass_utils

    bf16 = ml_dtypes.bfloat16
    X = np.asarray(X, dtype=np.float32)
    W_ih = np.asarray(W_ih, dtype=np.float32)
    hh = np.asarray(hh, dtype=np.float32)
    W_ho = np.asarray(W_ho, dtype=np.float32)
    b_ho = np.asarray(b_ho, dtype=np.float32)

    plan = _make_plan(hh)
    perm = plan["perm"]
    nc = _get_program(plan)

    wiht = np.ascontiguousarray(W_ih[perm].T).astype(bf16)  # [I, H]
    whot = np.ascontiguousarray(-W_ho[:, perm].T).astype(bf16)  # [H, O], negated
    bias = np.tile(b_ho[None, :], (BC, 1)).astype(np.float32)

    common = {"WIHT": wiht, "WHOT": whot, "BIAS": bias, "SC": plan["SC"]}
    in_maps = []
    for m in range(NCORES):
        im = dict(common)
        xm = X[:, m * BC : (m + 1) * BC, :]  # [S, BC, I]
        # device layout [NBLK, 128(i-within), (ic, b, tau)] — partition-major
        xt = xm.reshape(NBLK, TB, BC, NI, 128).transpose(0, 4, 3, 2, 1)
        im["X"] = (
            np.ascontiguousarray(xt).reshape(NBLK, 128, NI * BC * TB).astype(bf16)
        )
        in_maps.append(im)

    trace = bool(int(os.environ.get("DIAG_TRACE", "0")))
    if trace:
        trace = _ensure_ntff_hook()
    res = None
    for attempt in range(3):
        try:
            res = bass_utils.run_bass_kernel_spmd(
                nc,
                in_maps,
                core_ids=list(range(NCORES)),
                trace=trace,
                tmpdir=os.environ.get("DIAG_TRACE_DIR") or None,
            )
            break
        except Exception:
            if attempt == 2:
                raise
            trace = False  # retry without profiling
    if res.exec_time_ns is not None:
        kernel.last_exec_time_ns = res.exec_time_ns
        kernel.last_mean_exec_time_ns = res.mean_exec_time_ns
    Yfull = np.concatenate([r["Y"] for r in res.results], axis=0)
    return Yfull


kernel.last_exec_time_ns = None
kernel.last_mean_exec_time_ns = None


# revision 18
# speedup vs baseline: 3.9777x; 1.0856x over previous
"""Trainium2 Bass kernel for nn_Diagnet (S=1024, B=64, I=512, H=2048, O=512).

    u = einsum('sbi,hi->sbh', X, W_ih)
    h_t = |u_t + hh * h_{t-1}|   (scan over S, only final h needed)
    Y = h_final @ W_ho.T + b_ho

Strategy (8 NeuronCores, data-parallel over batch, 8 batch rows per core):

* H lanes are permuted so hh is sorted descending and split into 16
  chunks of 128.  With per-lane decay a = hh < 1, a chunk whose largest
  a satisfies a^K < 1e-10 only needs the last K steps (exact to ~1e-10,
  far below fp32 noise): the GEMM and the scan skip everything earlier.
* The scan runs as ONE custom DVE instruction per (chunk, 64-step
  block): a hand-tuned micro-op program (SCANDIAG_ANT) folds
  m_k = |m_{k-1} - x_k * sc_k| along the free dimension at 1 elem/cycle,
  with the 8 batch chains as sub-dimension pages that re-init the fold
  state to 0 at each page boundary.  State is kept pre-scaled
  (m = a^(K-1-t) h, scales folded into the sc stream), so the step is
  multiply-free in the recurrence itself and no inter-block rescale is
  needed.  Block-to-block state is chained by a carry element prepended
  to each page, written by the previous block's scan through a stride-0
  output AP (65 writes to one address; last write = final state).  The
  scan outputs -m so the carry re-seeds exactly: |0 - (-m)| = m.
* GEMM in bf16 (1 PE cycle/row vs 4 for fp32): X and W_ih^T are cast
  host-side.  PSUM drains to SBUF as plain copies on the otherwise-idle
  Activation engine; the scale (and its sign flip) ride the scan's
  second input stream.
* Final projection: h arrives in [128, (chunk, batch)] layout; 16
  accumulating bf16 matmuls against host-transposed-negated W_ho^T
  (negated because the scan hands over -h), bias added on DVE.
"""

import math
import os

from contextlib import ExitStack

import numpy as np

S, B, I, H, O = 1024, 64, 512, 2048, 512
NCORES = 8
BC = B // NCORES  # 8 batch rows per core
TB = 64  # time block
NBLK = S // TB  # 16
NCH = H // 128  # 16 h-chunks
NI = I // 128  # 4 i-chunks
NSLOT = TB + 1  # carry + 64 scan elements per page
LN_TRUNC = 23.03  # a^K <= e^-23 ~ 1e-10 -> truncate

_CACHE = {}


def _register_scandiag():
    """Custom DVE op: per page s of [P, S, N] in0 (in1 broadcast over pages):
        m = 0
        for k: m = |m - in0[p,s,k] * in1[p,k]| ; out[p,s,k] = -m
    Built from lower() of a Spec, with the step uop hand-edited so the
    fold state re-inits to 0 at each page boundary."""
    import concourse.dve_ops as dve_ops
    from concourse.dve_spec import Spec, Src0, Src1, Zero, scan, PageIdx, lower, AluOp
    from concourse.dve_uop import DveOpSpec, AluInp

    name = "SCANDIAG_ANT"
    for op in dve_ops.OPS:
        if op.name == name:
            return op

    body = (Zero - scan(AluOp.ABSOLUTE_DIFF, Src0 * Src1, init=Zero)) + PageIdx(
        Zero, Zero
    )

    def ref(in0, in1, s0, s1, imm2):
        x = in0.astype(np.float32)
        w = in1.astype(np.float32)
        if x.ndim == 2:
            x = x[:, None, :]
        if w.ndim == 3:
            w = w[:, 0, :]
        out = np.empty_like(x)
        for s in range(x.shape[1]):
            m = np.zeros(x.shape[0], dtype=np.float32)
            for k in range(x.shape[2]):
                m = np.abs(m - x[:, s, k] * w[:, k])
                out[:, s, k] = -m
        return out.reshape(in0.shape)

    spec = Spec(body=body, reference=ref)
    row = max(dve_ops._SUB_OPCODE_FOR_NAME.values()) + 1
    assert row < 0x20
    shas = {}
    compiled = {}
    for ver in ("v3", "v4"):
        uops = lower(spec, ver=ver)
        assert len(uops) == 3, (ver, len(uops))
        hits = [
            k
            for k, b in enumerate(uops[2].datapath_config)
            if b.op == AluOp.ABSOLUTE_DIFF and b.alu_src0 == AluInp.CURR_ALU_OUT
        ]
        assert len(hits) == 1, hits
        k = hits[0]
        seed_blk = uops[0].datapath_config[k]
        assert seed_blk.op == AluOp.BYPASS
        uops[2].datapath_config[k].alu_src0 = seed_blk.alu_src0
        s = DveOpSpec(name=name, opcode=row, uops=uops, rd1_en=True)
        shas[ver] = s.sha(ver)
        compiled[ver] = s
    op = dve_ops.DveOp(name, spec, subdim=True, uops_sha=shas)
    for ver in ("v3", "v4"):
        dve_ops._COMPILE_CACHE[(name, ver)] = compiled[ver]
    dve_ops._SUB_OPCODE_FOR_NAME[name] = row
    dve_ops.OPS.append(op)
    dve_ops.CUSTOM_DVE_SPECS[name] = spec
    return op


def _make_plan(hh):
    a = np.maximum(np.abs(hh.astype(np.float64)), 1e-30)
    perm = np.argsort(-a, kind="stable")
    a_s = a[perm]
    first_block = []
    for g in range(NCH):
        amax = a_s[g * 128]
        if amax >= math.exp(-LN_TRUNC / S):
            kg = S
        else:
            kg = min(S, int(math.ceil(LN_TRUNC / math.log(1.0 / amax))))
        kg = min(S, max(TB, ((kg + TB - 1) // TB) * TB))
        first_block.append(NBLK - kg // TB)
    assert all(
        first_block[g] <= first_block[g + 1] for g in range(NCH - 1)
    ), first_block

    # SC stream: one 65-entry segment per active (chunk, block), kb-major so
    # early-needed segments sit first (the DMA is split on this boundary).
    # slot 0 (carry) = 1.0 ; slot 1+tau = -a^(K-1 - ((kb-fb)*TB+tau))
    ag = a_s.reshape(NCH, 128).T  # [128, NCH]
    segs = [(g, kb) for kb in range(NBLK) for g in range(NCH) if first_block[g] <= kb]
    seg_off = {gk: i * NSLOT for i, gk in enumerate(segs)}
    sc = np.zeros((128, len(segs) * NSLOT), dtype=np.float64)
    tau = np.arange(TB)
    for (g, kb), off in seg_off.items():
        kg = (NBLK - first_block[g]) * TB
        e = kg - 1 - ((kb - first_block[g]) * TB + tau)  # [TB] exponents
        sc[:, off] = 1.0
        sc[:, off + 1 : off + NSLOT] = -(ag[:, g : g + 1] ** e[None, :])
    # early part: segments for blocks before the second chunk activates
    nearly = sum(1 for (g, kb) in segs if kb < first_block[1])
    return {
        "perm": perm,
        "first_block": tuple(first_block),
        "SC": sc,  # float64; cast at upload
        "nseg": len(segs),
        "nearly": max(nearly, 1),
        "seg_off": seg_off,
    }


def _build(first_block, nseg, nearly, seg_off):
    import concourse.mybir as mybir
    import concourse.tile as tile
    from concourse import bacc
    from concourse.bass import ds

    SCANDIAG = _register_scandiag()
    f32 = mybir.dt.float32
    bf16 = mybir.dt.bfloat16

    nc = bacc.Bacc("TRN2", target_bir_lowering=False, debug=False, num_devices=NCORES)
    X = nc.dram_tensor("X", [NBLK, 128, NI * BC * TB], bf16, kind="ExternalInput").ap()
    WIHT = nc.dram_tensor("WIHT", [I, H], bf16, kind="ExternalInput").ap()
    SC = nc.dram_tensor("SC", [128, nseg * NSLOT], bf16, kind="ExternalInput").ap()
    WHOT = nc.dram_tensor("WHOT", [H, O], bf16, kind="ExternalInput").ap()
    BIAS = nc.dram_tensor("BIAS", [BC, O], f32, kind="ExternalInput").ap()
    Y = nc.dram_tensor("Y", [BC, O], f32, kind="ExternalOutput").ap()

    acts = [[g for g in range(NCH) if first_block[g] <= kb] for kb in range(NBLK)]

    with tile.TileContext(nc) as tc:
        with ExitStack() as ctx:
            consts = ctx.enter_context(tc.tile_pool(name="consts", bufs=1))
            xtpool = ctx.enter_context(tc.tile_pool(name="xt", bufs=6))
            upool = ctx.enter_context(tc.tile_pool(name="ubuf", bufs=3))
            ypool = ctx.enter_context(tc.tile_pool(name="yout", bufs=1))
            gpool = ctx.enter_context(
                tc.tile_pool(name="gpsum", bufs=int(os.environ.get("DIAG_GP", "7")), space="PSUM")
            )
            fpool = ctx.enter_context(tc.tile_pool(name="fpsum", bufs=1, space="PSUM"))

            # --- constants ---
            # chunk-0 columns as a small separate tile: blocks 0..fb[1]-1 only
            # touch chunk 0, and this 128KB DMA unblocks the first GEMMs fast.
            wih0 = consts.tile([128, NI * 128], bf16, tag="wih0", name="wih0")
            nc.sync.dma_start(
                wih0[:].rearrange("p (i h) -> p i h", i=NI),
                WIHT.rearrange("(i p) h -> p i h", p=128)[:, :, 0:128],
            )
            sc_t = consts.tile([128, nseg * NSLOT], bf16, tag="sc", name="sc_t")
            ne = nearly * NSLOT
            nc.sync.dma_start(sc_t[:, 0:ne], SC[:, 0:ne])
            wiht = [
                consts.tile([128, H], bf16, tag=f"wiht{ic}", name=f"wiht{ic}")
                for ic in range(NI)
            ]
            hbuf = consts.tile([128, NCH * BC], bf16, tag="hbuf", name="hbuf")

            ucur = {}

            def wslice(g, ic):
                if g == 0:
                    return wih0[:].rearrange("p (i h) -> p i h", i=NI)[:, ic]
                return wiht[ic][:, ds(g * 128, 128)]

            for kb in range(NBLK):
                # X tiles for this block: one DMA, [128, (ic, b*t)]
                xt = xtpool.tile([128, NI * TB * BC], bf16, tag="xt", name=f"xt_{kb}")
                x3 = xt[:].rearrange("p (i n) -> p i n", i=NI)
                if kb == 0:
                    # split the first block across 4 DMAs so it lands sooner
                    xk = X[kb].rearrange("p (i n) -> p i n", i=NI)
                    for ic in range(NI):
                        nc.gpsimd.dma_start(x3[:, ic], xk[:, ic])
                else:
                    nc.gpsimd.dma_start(xt[:], X[kb])

                if kb == 2:
                    # rest of the scale table (not needed until block fb[1])
                    nc.sync.dma_start(sc_t[:, ne:], SC[:, ne:])
                if kb == 5:
                    # full W_ih columns: first needed at block first_block[1]
                    for ic in range(NI):
                        nc.sync.dma_start(wiht[ic][:], WIHT[ds(ic * 128, 128), :])
                if kb == 8:
                    # needed only at the end; issued here to overlap
                    whot = consts.tile([128, NCH * O], bf16, tag="whot", name="whot")
                    nc.sync.dma_start(
                        whot[:].rearrange("p (g o) -> p g o", g=NCH),
                        WHOT.rearrange("(g p) o -> p g o", p=128),
                    )
                    bias_t = ypool.tile([BC, O], f32, tag="bias", name="bias_t")
                    nc.sync.dma_start(bias_t[:], BIAS)

                for g in acts[kb]:
                    ps = gpool.tile([128, TB * BC], f32, tag="gp", name=f"gp_{kb}_{g}")
                    for ic in range(NI):
                        nc.tensor.matmul(
                            ps[:],
                            wslice(g, ic),
                            x3[:, ic],
                            start=(ic == 0),
                            stop=(ic == NI - 1),
                        )
                    if kb == first_block[g]:
                        u_t = upool.tile([128, BC * NSLOT], f32, tag=f"u{g}", name=f"u_{g}_{kb}")
                        nc.gpsimd.memset(
                            u_t[:].rearrange("p (s n) -> p s n", n=NSLOT)[:, :, 0:1], 0.0
                        )
                        ucur[g] = u_t
                    u3 = ucur[g][:].rearrange("p (s n) -> p s n", n=NSLOT)
                    # drain PSUM -> u slots 1..64 (plain copy on Activation;
                    # GPSIMD cannot read PSUM on TRN2)
                    ps3 = ps[:].rearrange("p (s n) -> p s n", s=BC)
                    nc.scalar.copy(u3[:, :, 1:NSLOT], ps3)
                    scs = sc_t[:, ds(seg_off[(g, kb)], NSLOT)].unsqueeze(1).broadcast_to(
                        [128, BC, NSLOT]
                    )
                    if kb < NBLK - 1:
                        u_nxt = upool.tile(
                            [128, BC * NSLOT], f32, tag=f"u{g}", name=f"u_{g}_{kb + 1}"
                        )
                        out_ap = (
                            u_nxt[:]
                            .rearrange("p (s n) -> p s n", n=NSLOT)[:, :, 0:1]
                            .broadcast_to([128, BC, NSLOT])
                        )
                    else:
                        out_ap = hbuf[:, ds(g * BC, BC)].broadcast_to([128, BC, NSLOT])
                    nc.vector._custom_dve(SCANDIAG, out=out_ap, in0=u3, in1=scs)
                    if kb < NBLK - 1:
                        ucur[g] = u_nxt

            # --- final projection: Y = (-h)^T @ (-W_ho^T) + bias ---
            psy = fpool.tile([BC, O], f32, tag="fy", name="psy")
            wh3 = whot[:].rearrange("p (g o) -> p g o", g=NCH)
            for g in range(NCH):
                nc.tensor.matmul(
                    psy[:],
                    hbuf[:, ds(g * BC, BC)],
                    wh3[:, g],
                    start=(g == 0),
                    stop=(g == NCH - 1),
                )
            y_t = ypool.tile([BC, O], f32, tag="y", name="y_t")
            nc.vector.tensor_tensor(y_t[:], psy[:], bias_t[:], mybir.AluOpType.add)
            nc.sync.dma_start(Y, y_t[:])
    nc.compile()
    return nc


def _get_program(plan):
    key = (plan["first_block"], os.environ.get("DIAG_GP"))
    if key not in _CACHE:
        _CACHE[key] = _build(
            plan["first_block"], plan["nseg"], plan["nearly"], plan["seg_off"]
        )
    return _CACHE[key]


def _ensure_ntff_hook():
    """Provide antenv.axon_hooks (absent in this image) so trace=True works."""
    import sys
    import types

    if "antenv.axon_hooks" in sys.modules:
        return True
    try:
        import antenv

        mod = types.ModuleType("antenv.axon_hooks")
        mod._hook = None

        def set_axon_ntff_profile_hook(h):
            mod._hook = h

        def get_axon_ntff_profile_hook():
            return mod._hook

        mod.set_axon_ntff_profile_hook = set_axon_ntff_profile_hook
        mod.get_axon_ntff_profile_hook = get_axon_ntff_profile_hook
        sys.modules["antenv.axon_hooks"] = mod
        antenv.axon_hooks = mod

        from trn_agent_boot.trn_boot import _ntff_profile_via_ctypes

        hook = _ntff_profile_via_ctypes("/opt/axon/libaxon_pjrt.so")
        mod.set_axon_ntff_profile_hook(hook)
        return hook is not None
    except Exception:
        return False


def kernel(X, W_ih, hh, W_ho, b_ho):
    import ml_dtypes
    from concourse import bass_utils

    bf16 = ml_dtypes.bfloat16
    X = np.asarray(X, dtype=np.float32)
    W_ih = np.asarray(W_ih, dtype=np.float32)
    hh = np.asarray(hh, dtype=np.float32)
    W_ho = np.asarray(W_ho, dtype=np.float32)
    b_ho = np.asarray(b_ho, dtype=np.float32)

    plan = _make_plan(hh)
    perm = plan["perm"]
    nc = _get_program(plan)

    wiht = np.ascontiguousarray(W_ih[perm].T).astype(bf16)  # [I, H]
    whot = np.ascontiguousarray(-W_ho[:, perm].T).astype(bf16)  # [H, O], negated
    bias = np.tile(b_ho[None, :], (BC, 1)).astype(np.float32)

    common = {
        "WIHT": wiht,
        "WHOT": whot,
        "BIAS": bias,
        "SC": plan["SC"].astype(bf16),
    }
    in_maps = []
    for m in range(NCORES):
        im = dict(common)
        xm = X[:, m * BC : (m + 1) * BC, :]  # [S, BC, I]
        # device layout [NBLK, 128(i-within), (ic, b, tau)] — partition-major
        xt = xm.reshape(NBLK, TB, BC, NI, 128).transpose(0, 4, 3, 2, 1)
        im["X"] = (
            np.ascontiguousarray(xt).reshape(NBLK, 128, NI * BC * TB).astype(bf16)
        )
        in_maps.append(im)

    trace = bool(int(os.environ.get("DIAG_TRACE", "0")))
    if trace:
        trace = _ensure_ntff_hook()
    res = None
    for attempt in range(3):
        try:
            res = bass_utils.run_bass_kernel_spmd(
                nc,
                in_maps,
                core_ids=list(range(NCORES)),
                trace=trace,
                tmpdir=os.environ.get("DIAG_TRACE_DIR") or None,
            )
            break
        except Exception:
            if attempt == 2:
                raise
            trace = False  # retry without profiling
    if res.exec_time_ns is not None:
        kernel.last_exec_time_ns = res.exec_time_ns
        kernel.last_mean_exec_time_ns = res.mean_exec_time_ns
    Yfull = np.concatenate([r["Y"] for r in res.results], axis=0)
    return Yfull


kernel.last_exec_time_ns = None
kernel.last_mean_exec_time_ns = None
